# revision 28
# baseline (speedup 1.0000x reference)
"""CQAttention (QANet context-query attention) Trainium2 kernel, v5 (bf16).

Full-input contract: kernel(**inputs) takes the unsharded arrays
  C [64, 1024, 256] f32, Q [64, 128, 256] f32,
  cmask [64, 1024] f32 (unused by the reference), qmask [64, 128] f32,
  w [768] f32
and returns out [64, 1024, 512] f32.

Sharding: batch dim across 8 NeuronCores (8 batches per core), no
cross-core communication.

Math notes (vs the reference):
  S[b,i,j] = C@w1 + Q@w2 + (C*w3)@Q^T, masked over j, softmax over j.
  - C@w1 is constant along the softmax axis j -> dropped (w1 unused).
  - q2 = Q@w2 is folded into the exp as a per-partition bias:
    bias = q2 - 1e4*qmask, so masked columns give exp(x-1e4) == 0.0
    exactly (underflow), identical to -1e30 mask + softmax.
  - Softmax denominator s[i] = sum_j E[j,i] via separate N=1 matmuls
    against a ones column, batched 4-per-PSUM-bank so one reciprocal
    op covers a half-batch.

Perf notes:
  - fp32r matmuls execute in fp32 HIGH (4-pump) mode on HW, so the
    whole matmul path is bf16 (1 cycle/row, FWL weight loads).
    rel err ~4e-3 vs the 2e-2 gate.
  - C^T via plain matmuls against a bf16 identity (~107ns spacing,
    counts as PE-busy for the HAM clock gate; transpose-mode does
    not and runs 2.5x slower).
  - Kernel is DMA-floor-bound: ~26MB @ ~360-400GB/s/core => ~70us.
    Everything else (engine schedule below) exists to keep the
    per-batch compute period at or below the store-drain period.
  - DMA: small inputs FIRST on the SP ring (Q batch 0 before all so
    qprep(0) unblocks at ~8us; 1KB-descriptor DMAs starve behind
    queued 8KB C loads in the DMA-engine round-robin). C loads 3
    deep, then pipelined b+3 (issuing all 8 up front exhausts the
    DMA semaphore pool and serializes issue at ~34us). Stores:
    batches 0-4 on the ACT ring, 5-7 on the then-idle SP ring, last
    batch in halves to shorten the drain tail.
  - Engine schedule per batch (measured ns budgets):
    ACT : cast C->bf16 (2x1.15u), exp (2x0.63), A-scale tt0/tt2
          (4x0.56), 1 ct-copy (0.69), store issue
    DVE : 3 ct-copies, recip (2x0.12), A-scale tt1/tt3, fused
          C*A=(U*r)*C from PSUM tt2/tt3, qT-copy, q2, bias
    POOL: C*A=A*C tt0/tt1 (SBUF only -- GPSIMD cannot touch PSUM),
          qw3T scale, q_rnd cast
"""

from contextlib import ExitStack

import numpy as np

import concourse.bacc as bacc
import concourse.bass as bass
import concourse.mybir as mybir
import concourse.tile as tile
from concourse.bass_utils import run_bass_kernel_spmd
from concourse.masks import make_identity

B, LC, LQ, D = 64, 1024, 128, 256
N_CORES = 8
BL = B // N_CORES  # batches per core
NT = LC // 128     # i-chunks per batch
KD = D // 128      # d-chunks (contraction tiles)
F32 = mybir.dt.float32
BF16 = mybir.dt.bfloat16
MULT = mybir.AluOpType.mult

_CACHE: dict = {}


def _build_bass() -> bass.Bass:
    nc = bacc.Bacc("TRN2")
    C_h = nc.dram_tensor("C", [BL, LC, D], F32, kind="ExternalInput")
    Q_h = nc.dram_tensor("Q", [BL, LQ, D], F32, kind="ExternalInput")
    qm_h = nc.dram_tensor("qmask", [BL, LQ], F32, kind="ExternalInput")
    w_h = nc.dram_tensor("w", [3 * D], F32, kind="ExternalInput")
    out_h = nc.dram_tensor("out", [BL, LC, 2 * D], F32, kind="ExternalOutput")

    with tile.TileContext(nc) as tc, ExitStack() as ctx:
        singles = ctx.enter_context(tc.tile_pool(name="singles", bufs=1))
        c_pool = ctx.enter_context(tc.tile_pool(name="c", bufs=BL))
        cb_pool = ctx.enter_context(tc.tile_pool(name="cb", bufs=2))
        ct_pool = ctx.enter_context(tc.tile_pool(name="ct", bufs=2))
        e_pool = ctx.enter_context(tc.tile_pool(name="e", bufs=2))
        # bufs=3: with 2, batch b+2's epilogue stalls on store(b)'s 2MB
        # drain (observed as a ~5us all-engine gap per batch)
        o_pool = ctx.enter_context(tc.tile_pool(name="o", bufs=3))
        small_pool = ctx.enter_context(tc.tile_pool(name="small", bufs=12))
        scratch_pool = ctx.enter_context(tc.tile_pool(name="scr", bufs=2))
        # PSUM budget (8 banks): ctp 2 + s 2 + u 3 + sd 1 = 8
        ctp_pool = ctx.enter_context(tc.tile_pool(name="ctp", bufs=2, space="PSUM"))
        s_pool = ctx.enter_context(tc.tile_pool(name="s", bufs=2, space="PSUM"))
        u_pool = ctx.enter_context(tc.tile_pool(name="u", bufs=3, space="PSUM"))
        sd_pool = ctx.enter_context(tc.tile_pool(name="sd", bufs=1, space="PSUM"))

        # ---------------- one-time setup ----------------
        ident32 = singles.tile([128, 128], F32)
        make_identity(nc, ident32)
        identb = singles.tile([128, 128], BF16)
        nc.vector.tensor_copy(out=identb, in_=ident32)
        one1 = singles.tile([1, 1], F32)
        nc.vector.memset(one1, 1.0)
        ones_row = singles.tile([1, 128], F32)
        nc.vector.memset(ones_row, 1.0)
        onescol = singles.tile([128, 1], BF16)
        nc.vector.memset(onescol, 1.0)

        # Load order on the SP ring: C batch 0 first (gates cast(0)),
        # then Q batch 0 (gates qprep(0)), then the other smalls. 1KB-
        # descriptor DMAs starve behind queued 8KB C loads in the DMA-
        # engine round-robin, so the smalls go before C batches 1-2.
        c_tiles = [None] * BL

        def load_c(b):
            c_t = c_pool.tile([128, NT, D], F32, name="c32")
            nc.sync.dma_start(
                out=c_t, in_=C_h[b].rearrange("(p t) d -> p t d", t=NT)
            )
            c_tiles[b] = c_t

        # batch 0's C in halves so cast(0,h0) can start ~2us earlier
        c0_t = c_pool.tile([128, NT, D], F32, name="c32")
        c_tiles[0] = c0_t
        for h in range(2):
            if h == 1:
                w_row = singles.tile([1, 3 * D], F32)
                nc.sync.dma_start(
                    out=w_row,
                    in_=bass.AP(tensor=w_h, offset=0, ap=[[1, 1], [1, 3 * D]]),
                )
            nc.sync.dma_start(
                out=c0_t[:, 4 * h : 4 * (h + 1), :],
                in_=bass.AP(
                    tensor=C_h,
                    offset=4 * h * D,
                    ap=[[NT * D, 128], [D, 4], [1, D]],
                ),
            )
            if h == 0:
                q_all = singles.tile([128, BL, D], F32)
                nc.sync.dma_start(
                    out=q_all[:, 0:1, :],
                    in_=bass.AP(
                        tensor=Q_h, offset=0, ap=[[D, 128], [LQ * D, 1], [1, D]]
                    ),
                )
        qm8 = singles.tile([BL, LQ], F32)
        nc.sync.dma_start(
            out=qm8, in_=bass.AP(tensor=qm_h, offset=0, ap=[[LQ, BL], [1, LQ]])
        )
        load_c(1)
        nc.sync.dma_start(
            out=q_all[:, 1:, :],
            in_=bass.AP(
                tensor=Q_h,
                offset=LQ * D,
                ap=[[D, 128], [LQ * D, BL - 1], [1, D]],
            ),
        )
        load_c(2)

        # w3T[p, k] = w[2D + 128k + p]; w2rep[p, :] = w2 broadcast
        wps = ctp_pool.tile([128, KD + D], F32, tag="ctp", name="wps")
        for k in range(KD):
            nc.tensor.matmul(
                wps[:, k : k + 1],
                w_row[:, 2 * D + 128 * k : 2 * D + 128 * (k + 1)],
                one1,
                start=True,
                stop=True,
            )
        nc.tensor.matmul(
            wps[:, KD:], ones_row, w_row[:, D : 2 * D], start=True, stop=True
        )
        w3T = singles.tile([128, KD], F32)
        nc.vector.tensor_copy(out=w3T, in_=wps[:, :KD])
        w2rep = singles.tile([128, D], F32)
        nc.vector.tensor_copy(out=w2rep, in_=wps[:, KD:])

        # qmT[j, b] = qmask[b, j] via one plain transpose-matmul
        qmT_ps = ctp_pool.tile([128, BL], F32, tag="ctp", name="qmT_ps")
        nc.tensor.matmul(qmT_ps, qm8, ident32[0:BL, 0:BL], start=True, stop=True)
        qmT = singles.tile([128, BL], F32)
        nc.vector.tensor_copy(out=qmT, in_=qmT_ps)


        # per-batch Q-side tiles. w3 is folded into the C^T copy (ct =
        # ctp * w3T rides free on the PSUM->SBUF cast), so the S matmul
        # uses plain Q^T as lhsT and no (Q*w3)^T tile exists at all.
        q_rnd = singles.tile([128, BL, D], BF16)        # Q_b bf16, rhs of U'
        qT_sb = singles.tile([128, BL, KD, 128], BF16)  # Q_b^T chunks
        bias_all = singles.tile([128, BL], F32)         # q2 - 1e4*qmask

        def qprep_a(b):
            """Q-side prep phase 1 for batch b: q_rnd cast + Q^T."""
            nc.gpsimd.tensor_copy(out=q_rnd[:, b], in_=q_all[:, b])  # cast
            qT_ps = ctp_pool.tile([128, KD, 128], F32, tag="ctp", name="qT_ps")
            for k in range(KD):
                nc.tensor.matmul(
                    qT_ps[:, k],
                    q_rnd[:, b, 128 * k : 128 * (k + 1)],
                    identb,
                    start=True,
                    stop=True,
                )
            nc.vector.tensor_copy(out=qT_sb[:, b], in_=qT_ps)  # cast to bf16

        def qprep_b(b):
            """Q-side prep phase 2: q2 = sum_d Q*w2 via fused mult + accum
            reduction, bias = q2 - 1e4*qmask (both DVE)."""
            q2sb = small_pool.tile([128, 1], F32, name="q2sb")
            scr = scratch_pool.tile([128, D], F32, name="scr")
            nc.vector.scalar_tensor_tensor(
                out=scr,
                in0=q_all[:, b],
                scalar=1.0,
                in1=w2rep,
                op0=MULT,
                op1=MULT,
                accum_out=q2sb,
            )
            nc.vector.scalar_tensor_tensor(
                out=bias_all[:, b : b + 1],
                in0=qmT[:, b : b + 1],
                scalar=-10000.0,
                in1=q2sb,
                op0=MULT,
                op1=mybir.AluOpType.add,
            )

        # ---------------- per-batch pipeline stages ----------------
        def cast_c(b, h):
            """c32 half -> bf16 on ACT."""
            if h == 0:
                cast_c.cb = cb_pool.tile([128, NT, D], BF16)
            cb_t = cast_c.cb
            nc.scalar.copy(
                out=cb_t[:, 4 * h : 4 * (h + 1), :],
                in_=c_tiles[b][:, 4 * h : 4 * (h + 1), :],
            )
            return cb_t

        def stage_a(b, cb_t):
            """C^T transposes -> S^T matmul -> exp -> E (bf16)."""
            ct_t = ct_pool.tile([128, KD, LC], BF16)
            # 4 groups of 4 transposes: (half h, k-chunk k)
            for g in range(4):
                h, k = g >> 1, g & 1
                ctp = ctp_pool.tile([128, 4, 128], F32, tag="ctp")
                for tt in range(4):
                    t = 4 * h + tt
                    nc.tensor.matmul(
                        ctp[:, tt],
                        cb_t[:, t, 128 * k : 128 * (k + 1)],
                        identb,
                        start=True,
                        stop=True,
                    )
                # PSUM f32 -> SBUF bf16 copy-cast with the w3 scale folded
                # in as a per-partition scalar (3 DVE, 1 ACT)
                dst = ct_t[:, k, 512 * h : 512 * (h + 1)]
                if g == 3:
                    nc.scalar.mul(out=dst, in_=ctp, mul=w3T[:, k : k + 1])
                else:
                    nc.vector.tensor_scalar_mul(
                        out=dst, in0=ctp, scalar1=w3T[:, k : k + 1]
                    )

            e_t = e_pool.tile([128, LC], BF16)
            for h in range(2):
                s_t = s_pool.tile([128, 512], F32, tag="s")
                for k in range(KD):
                    nc.tensor.matmul(
                        s_t,
                        qT_sb[:, b, k],
                        ct_t[:, k, 512 * h : 512 * (h + 1)],
                        start=(k == 0),
                        stop=(k == KD - 1),
                    )
                nc.scalar.activation(
                    out=e_t[:, 512 * h : 512 * (h + 1)],
                    in_=s_t,
                    func=mybir.ActivationFunctionType.Exp,
                    bias=bias_all[:, b : b + 1],
                    scale=1.0,
                )
            return e_t

        def stage_b_half(b, e_t, o_t, h):
            """Half-batch epilogue: U' matmuls + denominators, one recip,
            A-scale and C*A per chunk."""
            c_t = c_tiles[b]
            u_ts = []
            sd_t = sd_pool.tile([128, 4], F32, tag="sd", name="sd_t")
            for tt in range(4):
                t = 4 * h + tt
                if tt % 2 == 0:
                    u_t = u_pool.tile([128, 2, D], F32, tag="u")
                    u_ts.append(u_t)
                e_ch = e_t[:, 128 * t : 128 * (t + 1)]
                nc.tensor.matmul(
                    u_ts[-1][:, tt % 2], e_ch, q_rnd[:, b], start=True, stop=True
                )
                nc.tensor.matmul(
                    sd_t[:, tt : tt + 1], e_ch, onescol, start=True, stop=True
                )
            r4 = small_pool.tile([128, 4], F32)
            nc.vector.reciprocal(out=r4, in_=sd_t)
            for tt in range(4):
                t = 4 * h + tt
                u_ch = u_ts[tt // 2][:, tt % 2]
                r_t = r4[:, tt : tt + 1]
                # A = U*r: 1 ACT + 3 DVE per half (PSUM read: ACT/DVE only)
                if tt == 0:
                    nc.scalar.mul(out=o_t[:, t, :D], in_=u_ch, mul=r_t)
                else:
                    nc.vector.tensor_scalar_mul(
                        out=o_t[:, t, :D], in0=u_ch, scalar1=r_t
                    )
                # C*A: 3 POOL (A*C, SBUF only) + 1 DVE fused from PSUM
                if tt < 3:
                    nc.gpsimd.tensor_mul(
                        o_t[:, t, D:], o_t[:, t, :D], c_t[:, t, :]
                    )
                else:
                    nc.vector.scalar_tensor_tensor(
                        out=o_t[:, t, D:],
                        in0=u_ch,
                        scalar=r_t,
                        in1=c_t[:, t, :],
                        op0=MULT,
                        op1=MULT,
                    )

        def store_o_half(b, o_t, h):
            """Store a half batch; late batches ride the idle SP ring."""
            ring = nc.scalar if b < 5 else nc.sync
            ring.dma_start(
                out=bass.AP(
                    tensor=out_h,
                    offset=b * LC * 2 * D + 4 * h * 2 * D,
                    ap=[[NT * 2 * D, 128], [2 * D, 4], [1, 2 * D]],
                ),
                in_=o_t[:, 4 * h : 4 * (h + 1), :],
            )

        # ---------------- software-pipelined emission ----------------
        # iter b: [load(b+3); cast-h0(b+1); B(b,h0); cast-h1(b+1); B(b,h1);
        #          store(b); qprep_a(b+2); qprep_b(b+1); A(b+1)]
        qprep_a(0)
        cb = cast_c(0, 0)
        cast_c(0, 1)
        qprep_b(0)
        e_cur = stage_a(0, cb)
        qprep_a(1)
        for b in range(BL):
            if b + 3 < BL:
                load_c(b + 3)
            o_t = o_pool.tile([128, NT, 2 * D], F32)
            cb_nxt = cast_c(b + 1, 0) if b + 1 < BL else None
            stage_b_half(b, e_cur, o_t, 0)
            store_o_half(b, o_t, 0)
            if b + 1 < BL:
                cast_c(b + 1, 1)
            stage_b_half(b, e_cur, o_t, 1)
            store_o_half(b, o_t, 1)
            if b + 2 < BL:
                qprep_a(b + 2)
            if b + 1 < BL:
                qprep_b(b + 1)
                e_cur = stage_a(b + 1, cb_nxt)
    nc.compile()
    return nc


def _get_bass() -> bass.Bass:
    if "nc" not in _CACHE:
        _CACHE["nc"] = _build_bass()
    return _CACHE["nc"]


def _run(C, Q, qmask, w, trace=False, **spmd_kwargs):
    nc = _get_bass()
    C = np.ascontiguousarray(C, dtype=np.float32)
    Q = np.ascontiguousarray(Q, dtype=np.float32)
    qmask = np.ascontiguousarray(qmask, dtype=np.float32)
    w = np.ascontiguousarray(w, dtype=np.float32)
    in_maps = [
        {
            "C": C[c * BL : (c + 1) * BL],
            "Q": Q[c * BL : (c + 1) * BL],
            "qmask": qmask[c * BL : (c + 1) * BL],
            "w": w,
        }
        for c in range(N_CORES)
    ]
    res = run_bass_kernel_spmd(
        nc, in_maps, list(range(N_CORES)), trace=trace, **spmd_kwargs
    )
    out = np.concatenate([res.results[c]["out"] for c in range(N_CORES)], axis=0)
    return out, res


def kernel(C, Q, cmask, qmask, w):
    out, _ = _run(C, Q, qmask, w, trace=False)
    return out
